# revision 29
# baseline (speedup 1.0000x reference)
"""Bilinear interaction kernel for Trainium2 (8 NeuronCores, SPMD).

Problem: inputs (32, 4096, 1, 64) f32 stacked field embeddings,
W (496, 64, 64) one bilinear weight per field pair (i<j).
out[b, p] = x_i[b] @ W_p @ x_j[b]   -> (4096, 496) f32.

Strategy (data-parallel over batch per the sharding hint):
 - shard batch 4096 -> 8 cores x 512 rows; W replicated.
 - host-side pre-layout: x passed batch-major (xn) for stage 2 and
   k-on-partition packed (xtp) for stage-1 stationaries; W packed into
   [128, cols] blocks pairing a partitions-0:63 chunk with a 64:127
   chunk, so stage-1 matmuls run row-packed (2 concurrent K=64 matmuls
   via tile_position auto-derived from base_partition).
 - stage 1 (PE): for each first-field i, T[b, (j,l)] = x_i @ W_i-block,
   psum chunks of <=512 cols.
 - stage 2 (DVE): M = T * xn[:, (j,l)-window] (second-field values line
   up because pairs with first field i have j in (i, 31] contiguous),
   then segmented reduce over l (64) -> out columns.

walrus allows ONE sync wait per instruction, so: no SBUF tile is ever
reused (no WAR waits on DMAs), and every DMA is followed by a cheap
"touch" op on its consumer engine (tiny matmul into a write-only psum
sink / tiny DVE copy) that absorbs the DMA-queue wait into that
engine's observed vector clock.
"""

import os
import sys

import numpy as np

sys.path.insert(0, "/opt/trn_rl_repo")

import concourse.bass as bass
import concourse.tile as tile
from concourse import mybir
from concourse.bass_utils import run_bass_kernel_spmd
from concourse.tile import ScopedClock


def _split_drain_and_barrier(self, tick_clock, wait_clock):
    """Replacement for TileContext._drain_and_barrier: walrus codegen
    accepts only one sync wait per instruction in this toolchain, but the
    kernel-tail drain collects one wait per active processor (engines +
    DMA queues, ~14 here). Emit one drain per wait instead."""
    drains = [self.nc.sync.drain() for _ in range(20)]
    wait_clock.add_sem_waits(
        drains[-1].ins, ScopedClock({None: tick_clock.global_clock})
    )
    si = drains[-1].ins.sync_info
    ow = list(si.on_wait) if si is not None and si.on_wait else []
    if len(ow) > 1:
        for d, w in zip(drains[:-1], ow[:-1]):
            d.ins.sync_info = mybir.SyncInfo(on_wait=[w], on_update=[])
        drains[-1].ins.sync_info = mybir.SyncInfo(
            on_wait=[ow[-1]],
            on_update=list(si.on_update) if si.on_update else [],
        )

    self.nc.all_engine_barrier()
    assert self.sems is not None
    popped = self.nc._tile_sem_poison_stack.pop()
    assert popped is self._sem_poison
    self.nc.clear_and_free_semaphores(list(self.sems.allocated().values()))
    self.nc.all_engine_barrier()


tile.TileContext._drain_and_barrier = _split_drain_and_barrier

if os.environ.get("BILINEAR_MODE", "f32") == "bf16":
    # the identity-reduce bursts reload the same stationary every matmul;
    # let walrus elide the redundant LDWEIGHTS
    import concourse.bass_utils as _bu

    _orig_walrus_args = _bu.get_walrus_args

    def _walrus_args_ldwopt(*a, **kw):
        args = _orig_walrus_args(*a, **kw)
        return [
            x.replace("--enable-ldw-opt=false", "--enable-ldw-opt=true")
            if isinstance(x, str) else x
            for x in args
        ]

    _bu.get_walrus_args = _walrus_args_ldwopt

NF = 32          # fields
B = 4096         # total batch
K = 64           # embedding dim
P = NF * (NF - 1) // 2   # 496 pairs
NCORES = 8
BC = B // NCORES          # 512 rows per core
BT = 128                  # batch tile (partition dim)
NBT = BC // BT            # 4 batch tiles per core
F32 = mybir.dt.float32
BF16 = mybir.dt.bfloat16
CHUNK = 512
GP_OF_8 = int(os.environ.get("BILINEAR_GP_OF_8", "0"))  # chunks of 8 routed to GPSIMD
# "f32": exact, DVE does multiply+reduce.
# "bf16": products rounded to bf16; PE identity-matmuls accumulate the
#         reduction in PSUM (f32), roughly halving the DVE-bound runtime.
MODE = os.environ.get("BILINEAR_MODE", "f32")
MTP = 124          # pairs per M tile in bf16 mode (496 = 4 * 124)

# pair-group column offsets (pairs ordered like itertools.combinations)
_GRP_OFF = [0] * NF
for _i in range(1, NF):
    _GRP_OFF[_i] = _GRP_OFF[_i - 1] + (NF - _i)

# ---- field -> (xt tile, slot, half) packing ---------------------------------
# top half (partitions 0:64): fields 0-7 (tile A slots 0-7) and 24-31 (tile B)
# bottom half (64:128):       fields 8-15 (tile A) and 16-23 (tile B)
# Chunk-count per half balances 38/38.


def _field_loc(i: int):
    if i < 8:
        return 0, i, 0        # tile A, slot i, top
    if i < 16:
        return 0, i - 8, 1    # tile A, bottom
    if i < 24:
        return 1, i - 16, 1   # tile B, bottom
    return 1, i - 24, 0       # tile B, top

# host-side xtp[p, (tile, slot), b]: p<64 -> top fields, p>=64 -> bottom
_XTP_FIELD = np.zeros((2, 16), dtype=np.int64)   # [half, tile*8+slot] -> field
for _i in range(NF):
    _tl, _sl, _hf = _field_loc(_i)
    _XTP_FIELD[_hf, _tl * 8 + _sl] = _i

# ---- stage chunks and W block packing ---------------------------------------
# chunk = (i, off, w): psum tile of w<=512 T columns for first-field i
_CHUNKS = []
for _i in range(NF - 1):
    _ncols = (NF - 1 - _i) * K
    for _off in range(0, _ncols, CHUNK):
        _CHUNKS.append((_i, _off, min(CHUNK, _ncols - _off)))

_top = [c for c in _CHUNKS if _field_loc(c[0])[2] == 0]
_bot = [c for c in _CHUNKS if _field_loc(c[0])[2] == 1]
_top.sort(key=lambda c: -c[2])
_bot.sort(key=lambda c: -c[2])
assert len(_top) == len(_bot) == 38

# W blocks: block j pairs _top[j] (rows 0:64) with _bot[j] (rows 64:128).
_WBLK = []        # (wtop_chunk, wbot_chunk, width)
_WCOL = []        # start col of block j in packed W
_c = 0
for _j in range(38):
    _w = max(_top[_j][2], _bot[_j][2])
    _WBLK.append((_top[_j], _bot[_j], _w))
    _WCOL.append(_c)
    _c += _w
_WCOLS = _c

# execution order: alternate top/bottom so PE row-packing overlaps
_ORDER = []       # (chunk, blk_idx, half)
for _j in range(38):
    _ORDER.append((_top[_j], _j, 0))
    _ORDER.append((_bot[_j], _j, 1))

# chunk -> (block, half) for order-independent W lookup
_CHUNK_LOC = {}
for _j, (_ct, _cb, _w) in enumerate(_WBLK):
    _CHUNK_LOC[(_ct[0], _ct[1])] = (_j, 0)
    _CHUNK_LOC[(_cb[0], _cb[1])] = (_j, 1)


def _build_module() -> bass.Bass:
    if MODE == "bf16":
        return _build_module_bf16()
    nc = bass.Bass()
    xn = nc.declare_dram_parameter("xn", [BC, NF * K], F32, isOutput=False)
    xtp = nc.declare_dram_parameter("xtp", [BT, 16, BC], F32, isOutput=False)
    wt = nc.declare_dram_parameter("wt", [BT, _WCOLS], F32, isOutput=False)
    outs = [
        nc.declare_dram_parameter(f"out{t}", [BT, P], F32, isOutput=True)
        for t in range(NBT)
    ]

    with tile.TileContext(nc) as tc:
        with (
            tc.tile_pool(name="wpool", bufs=1) as wpool,
            tc.tile_pool(name="xnp", bufs=1) as xnp,
            tc.tile_pool(name="xtp_pool", bufs=1) as xtpool,
            tc.tile_pool(name="mp", bufs=4) as mp,
            tc.tile_pool(name="mgp", bufs=4) as mgp,
            tc.tile_pool(name="tstp", bufs=4) as tstp,
            tc.tile_pool(name="accp", bufs=1) as accp,
            tc.tile_pool(name="psum", bufs=6, space=bass.MemorySpace.PSUM) as psum,
            tc.tile_pool(name="sinkp", bufs=1, space=bass.MemorySpace.PSUM) as sinkp,
        ):
            sink = sinkp.tile([BT, 8], F32)
            junk = wpool.tile([BT, 256], F32, tag="junk")
            junka = wpool.tile([BT, 256], F32, tag="junka")
            junkg = wpool.tile([BT, 256], F32, tag="junkg")
            _tcnt = {"d": 0, "a": 0, "g": 0}

            def pe_touch(ap2d):
                # tiny matmul into the write-only sink: folds the DMA queue
                # semaphore into PE's observed clock (sink reuse is WAW on
                # PE itself -> no semaphore)
                nc.tensor.matmul(
                    sink[0:1, 0:1], ap2d[:, 0:1], ap2d[:, 0:1],
                    start=True, stop=True,
                )

            def dve_touch(ap2d):
                c = _tcnt["d"] % 256
                _tcnt["d"] += 1
                nc.vector.tensor_copy(junk[0:1, c:c + 1], ap2d[0:1, 0:1])

            def act_touch(ap2d):
                # absorb a foreign-engine tick into ACT's observed clock
                c = _tcnt["a"] % 256
                _tcnt["a"] += 1
                nc.scalar.copy(junka[0:1, c:c + 1], ap2d[0:1, 0:1])

            def gp_touch(ap2d):
                c = _tcnt["g"] % 256
                _tcnt["g"] += 1
                nc.gpsimd.tensor_copy(junkg[0:1, c:c + 1], ap2d[0:1, 0:1])

            gp_state = {"n": 0, "mg": [None] * 4, "acc": [None] * 4}

            # ---- W blocks: one DMA each, PE touches for both halves -----
            w_tiles = []
            for j, (ct, cb, w) in enumerate(_WBLK):
                wtile = wpool.tile([BT, w], F32, tag=f"w{j}")
                nc.sync.dma_start(wtile[:], wt[:, _WCOL[j]: _WCOL[j] + w])
                pe_touch(wtile[0:64, :])
                pe_touch(wtile[64:128, :])
                w_tiles.append(wtile)

            for t in range(NBT):
                xn_sb = xnp.tile([BT, NF * K], F32, tag=f"xn{t}")
                nc.sync.dma_start(xn_sb[:], xn[t * BT:(t + 1) * BT, :])
                dve_touch(xn_sb)

                xt_sb = []
                for tl in range(2):
                    xg = xtpool.tile([BT, 8, BT], F32, tag=f"xt{t}_{tl}")
                    nc.sync.dma_start(
                        xg[:], xtp[:, 8 * tl: 8 * (tl + 1), t * BT:(t + 1) * BT]
                    )
                    pe_touch(xg[0:64, 0, :])
                    pe_touch(xg[64:128, 0, :])
                    xt_sb.append(xg)

                gp_touch(xn_sb)

                acc = accp.tile([BT, P], F32, tag=f"acc{t}")

                for ci, ((i, off, w), j, half) in enumerate(_ORDER):
                    g = w // K
                    tl, slot, hf = _field_loc(i)
                    assert hf == half
                    pb = 64 * half
                    ps = psum.tile([BT, CHUNK], F32)
                    nc.tensor.matmul(
                        ps[:, :w],
                        xt_sb[tl][pb: pb + 64, slot, :],        # [64,128] stat
                        w_tiles[j][pb: pb + 64, :w],            # [64,w] moving
                        start=True,
                        stop=True,
                    )
                    pcol = _GRP_OFF[i] + off // K
                    xn_win = xn_sb[:, (i + 1) * K + off: (i + 1) * K + off + w]
                    if ci % 8 < GP_OF_8:
                        # GPSIMD route: ACT stages T into SBUF, GPSIMD does
                        # the multiply, DVE reduces. Touch ops below clear
                        # the cross-engine WAR semaphores so every real op
                        # keeps a single sync wait.
                        s = gp_state
                        sl = s["n"] % 4
                        if s["mg"][sl] is not None:
                            act_touch(s["mg"][sl])       # GP mult(n-4) done
                            gp_touch(s["acc"][sl])       # reduce(n-4) done
                        tst = tstp.tile([BT, CHUNK], F32)
                        nc.scalar.copy(tst[:, :w], ps[:, :w])
                        mg = mgp.tile([BT, CHUNK // K, K], F32)
                        nc.gpsimd.tensor_mul(
                            mg[:, :g, :].rearrange("p a b -> p (a b)"),
                            tst[:, :w],
                            xn_win,
                        )
                        nc.vector.reduce_sum(
                            acc[:, pcol: pcol + g],
                            mg[:, :g, :],
                            axis=mybir.AxisListType.X,
                        )
                        s["mg"][sl] = mg[0:1, 0, 0:1]
                        s["acc"][sl] = acc[0:1, pcol: pcol + 1]
                        s["n"] += 1
                    else:
                        m = mp.tile([BT, CHUNK // K, K], F32)
                        nc.vector.tensor_mul(
                            m[:, :g, :].rearrange("p a b -> p (a b)"),
                            ps[:, :w],
                            xn_win,
                        )
                        nc.vector.reduce_sum(
                            acc[:, pcol: pcol + g],
                            m[:, :g, :],
                            axis=mybir.AxisListType.X,
                        )

                nc.gpsimd.dma_start(outs[t][:], acc[:])
    return nc


def _build_module_bf16() -> bass.Bass:
    """bf16-M variant: DVE multiplies T (psum f32) by xn, writing bf16
    products M; the PE reduces each 124-pair M tile with 64 accumulating
    identity matmuls into a per-b-tile PSUM acc bank (f32 accumulation, so
    the only precision loss is the bf16 rounding of the products)."""
    nc = bass.Bass()
    xn = nc.declare_dram_parameter("xn", [BC, NF * K], F32, isOutput=False)
    xtp = nc.declare_dram_parameter("xtp", [BT, 16, BC], F32, isOutput=False)
    wt = nc.declare_dram_parameter("wt", [BT, _WCOLS], F32, isOutput=False)
    ident = nc.declare_dram_parameter("ident", [BT, BT], BF16, isOutput=False)
    outs = [
        nc.declare_dram_parameter(f"out{t}", [BT, P], F32, isOutput=True)
        for t in range(NBT)
    ]

    with tile.TileContext(nc) as tc:
        with (
            tc.tile_pool(name="wpool", bufs=1) as wpool,
            tc.tile_pool(name="xnp", bufs=1) as xnp,
            tc.tile_pool(name="xtp_pool", bufs=1) as xtpool,
            tc.tile_pool(name="mbp", bufs=2) as mbp,
            tc.tile_pool(name="psum", bufs=3, space=bass.MemorySpace.PSUM) as psum,
            tc.tile_pool(name="accpsp", bufs=1, space=bass.MemorySpace.PSUM) as accpsp,
            tc.tile_pool(name="sinkp", bufs=1, space=bass.MemorySpace.PSUM) as sinkp,
        ):
            sink = sinkp.tile([BT, 8], F32)
            junk = wpool.tile([BT, 256], F32, tag="junk")
            _tcnt = {"d": 0}

            def pe_touch(ap2d):
                nc.tensor.matmul(
                    sink[0:1, 0:1], ap2d[:, 0:1], ap2d[:, 0:1],
                    start=True, stop=True,
                )

            def dve_touch(ap2d):
                c = _tcnt["d"] % 256
                _tcnt["d"] += 1
                nc.vector.tensor_copy(junk[0:1, c:c + 1], ap2d[0:1, 0:1])

            ident_sb = wpool.tile([BT, BT], BF16, tag="ident")
            nc.sync.dma_start(ident_sb[:], ident[:])
            pe_touch(ident_sb)

            w_tiles = []
            for j, (ct, cb, w) in enumerate(_WBLK):
                wtile = wpool.tile([BT, w], F32, tag=f"w{j}")
                nc.sync.dma_start(wtile[:], wt[:, _WCOL[j]: _WCOL[j] + w])
                pe_touch(wtile[0:64, :])
                pe_touch(wtile[64:128, :])
                w_tiles.append(wtile)

            for t in range(NBT):
                xn_sb = xnp.tile([BT, NF * K], F32, tag=f"xn{t}")
                nc.sync.dma_start(xn_sb[:], xn[t * BT:(t + 1) * BT, :])
                dve_touch(xn_sb)

                xt_sb = []
                for tl in range(2):
                    xg = xtpool.tile([BT, 8, BT], F32, tag=f"xt{t}_{tl}")
                    nc.sync.dma_start(
                        xg[:], xtp[:, 8 * tl: 8 * (tl + 1), t * BT:(t + 1) * BT]
                    )
                    pe_touch(xg[0:64, 0, :])
                    pe_touch(xg[64:128, 0, :])
                    xt_sb.append(xg)

                acc_ps = accpsp.tile([BT, CHUNK], F32, tag=f"accps{t}")

                cur = {"mt": -1, "tile": None}
                ps_hist = []

                def flush(cur=cur, acc_ps=acc_ps, ident_sb=ident_sb):
                    if cur["tile"] is None:
                        return
                    pe_touch(cur["tile"][0:64, 0, :])
                    base = cur["mt"] * MTP
                    for l in range(K):
                        nc.tensor.matmul(
                            acc_ps[:, base: base + MTP],
                            ident_sb[:, :],
                            cur["tile"][:, :, l],
                            start=(l == 0),
                            stop=(l == K - 1),
                        )
                    cur["tile"] = None

                for (i, off, w) in _CHUNKS:
                    j, half = _CHUNK_LOC[(i, off)]
                    g = w // K
                    tl, slot, hf = _field_loc(i)
                    assert hf == half
                    pb = 64 * half
                    if len(ps_hist) >= 3:
                        pe_touch(ps_hist.pop(0))
                    ps = psum.tile([BT, CHUNK], F32)
                    nc.tensor.matmul(
                        ps[:, :w],
                        xt_sb[tl][pb: pb + 64, slot, :],
                        w_tiles[j][pb: pb + 64, :w],
                        start=True,
                        stop=True,
                    )
                    pcol = _GRP_OFF[i] + off // K
                    # split at M-tile boundaries (fixed 124-pair ranges)
                    done = 0
                    while done < g:
                        pc = pcol + done
                        mt = pc // MTP
                        mloc = pc - mt * MTP
                        gg = min(g - done, MTP - mloc)
                        if mt != cur["mt"]:
                            flush()
                            cur["mt"] = mt
                            cur["tile"] = mbp.tile([BT, MTP, K], BF16, tag="mtile", name="mtile")
                        o = done * K
                        nc.vector.tensor_mul(
                            cur["tile"][:, mloc: mloc + gg, :]
                            .rearrange("p a b -> p (a b)"),
                            ps[:, o: o + gg * K],
                            xn_sb[:, (i + 1) * K + off + o:
                                  (i + 1) * K + off + o + gg * K],
                        )
                        done += gg
                        last_m = cur["tile"][0:64, mloc, :]
                    ps_hist.append(last_m)
                flush()

                acc_sb = xnp.tile([BT, P], F32, tag=f"accsb{t}", name="acc_sb")
                nc.scalar.copy(acc_sb[:], acc_ps[:, :P])
                nc.gpsimd.dma_start(outs[t][:], acc_sb[:])
    return nc


_NC_CACHE: dict[str, bass.Bass] = {}


def _get_module() -> bass.Bass:
    if "nc" not in _NC_CACHE:
        _NC_CACHE["nc"] = _build_module()
    return _NC_CACHE["nc"]


def _make_in_maps(inputs: np.ndarray, W: np.ndarray):
    x = np.ascontiguousarray(np.asarray(inputs, dtype=np.float32)[:, :, 0, :])
    W = np.asarray(W, dtype=np.float32)

    # packed W: block j = [top chunk | bottom chunk] on partition halves
    wt_host = np.zeros((BT, _WCOLS), dtype=np.float32)
    wt_flat = np.ascontiguousarray(W.transpose(1, 0, 2)).reshape(K, P * K)
    for j, (ct, cb, w) in enumerate(_WBLK):
        for half, (i, off, cw) in ((0, ct), (1, cb)):
            base = _GRP_OFF[i] * K + off
            wt_host[64 * half: 64 * half + 64, _WCOL[j]: _WCOL[j] + cw] = \
                wt_flat[:, base: base + cw]

    in_maps = []
    for c in range(NCORES):
        xs = x[:, c * BC:(c + 1) * BC, :]                      # (32, 512, 64)
        xn_host = np.ascontiguousarray(xs.transpose(1, 0, 2)).reshape(BC, NF * K)
        # xtp[p, slot16, b]: p<64 top fields, p>=64 bottom fields, k = p % 64
        xtp_host = np.empty((BT, 16, BC), dtype=np.float32)
        xt_all = xs.transpose(2, 0, 1)                         # (64, 32, 512)
        xtp_host[0:64] = xt_all[:, _XTP_FIELD[0], :]
        xtp_host[64:128] = xt_all[:, _XTP_FIELD[1], :]
        im = {"xn": xn_host, "xtp": xtp_host, "wt": wt_host}
        if MODE == "bf16":
            import ml_dtypes
            im["ident"] = np.eye(BT, dtype=ml_dtypes.bfloat16)
        in_maps.append(im)
    return in_maps


def kernel(inputs: np.ndarray, W: np.ndarray) -> np.ndarray:
    in_maps = _make_in_maps(inputs, W)
    nc = _get_module()
    res = run_bass_kernel_spmd(nc, in_maps, list(range(NCORES))).results
    return np.concatenate(
        [r[f"out{t}"] for r in res for t in range(NBT)], axis=0
    )


def kernel_profiled(inputs: np.ndarray, W: np.ndarray, tmpdir: str | None = None):
    """Run with NTFF tracing; returns (output, BassKernelResults)."""
    in_maps = _make_in_maps(inputs, W)
    nc = _get_module()
    br = run_bass_kernel_spmd(
        nc, in_maps, list(range(NCORES)), trace=True, tmpdir=tmpdir
    )
    out = np.concatenate(
        [r[f"out{t}"] for r in br.results for t in range(NBT)], axis=0
    )
    return out, br
